# revision 33
# baseline (speedup 1.0000x reference)
"""MoE (top-2 of 8 experts) forward on 8 Trainium2 NeuronCores.

Expert-parallel: core c owns expert c (w1[c], w2[c] bf16, SBUF-resident,
loaded with two big host-rearranged DMAs). Every core computes the full
routing on device (fp32 transposes in PE transpose mode, per-tile logits,
batched top-2 algebra, batched prefix matmuls). Dispatch compaction is
done entirely on-chip with one-hot selection matmuls (no DRAM
scatter/merge round trip); the compact MLP (capacity 640 = obs max 527
+ margin) runs in two PSUM waves with wide matmuls; weighted rows
scatter into an AllToAll send buffer laid out [dest_slab, CAP]; a slim
raw-bass tail (walrus here cannot compile collectives inside
TileContext) runs the a2a and the owner-side combine with offsets
precomputed in the Tile section.

kernel(**inputs) -> full [2048, 768] float32 output.
"""
import os
import sys

sys.path.insert(0, "/opt/trn_rl_repo")

import numpy as np

DEBUG = os.environ.get("MOE_DEBUG", "0") == "1"

import concourse.bass as bass
import concourse.mybir as mybir
import concourse.tile as tile
from concourse.bass import IndirectOffsetOnAxis

F32 = mybir.dt.float32
BF16 = mybir.dt.bfloat16
I32 = mybir.dt.int32
U32 = mybir.dt.uint32
AF = mybir.ActivationFunctionType
OP = mybir.AluOpType
AX = mybir.AxisListType

T, H, E, K, F = 2048, 768, 8, 2, 3072
P = 128
NCORE = 8
NT = T // P          # 16 token tiles
NH = H // P          # 6 hidden chunks
NF = F // P          # 24 ffn chunks
C = 640              # compact-list capacity per expert (mean 512, obs max 527)
NC = C // P          # 5 compact tiles
CAP = 96             # capacity per (expert, slab) cell (mean 64, obs max 82)
SEND_ROWS = NCORE * CAP          # 768 rows in the a2a payload
SEND_FULL = 1024                 # send buffer incl. trash rows
BIG = 8192.0
SLAB = T // NCORE    # 256 tokens per output slab
WAVES = [(0, 3), (3, 5)]  # MLP compact-tile waves (PSUM-bank limited)

# ---------------------------------------------------------------------------
# This container's walrus cannot attach sem-wait commands to most
# instruction types. Two workarounds (see _split_attached_waits and the
# patched kernel-tail below): waits are moved onto standalone
# EventSemaphore instructions, and the Tile tail drain's waits are
# split across a chain of SP nops.
_MAX_WAITS = 4


def _patched_drain_and_barrier(self, tick_clock, wait_clock):
    from concourse.tile import ScopedClock, VectorClock
    from concourse.tile_sem_assignment import N_PROCS

    g = tick_clock.global_clock
    ticks = [g[p] for p in range(N_PROCS)]
    procs = [p for p in range(N_PROCS) if ticks[p] > 0]
    observed = [0] * N_PROCS
    for i in range(0, len(procs), _MAX_WAITS):
        chunk = set(procs[i : i + _MAX_WAITS])
        part = VectorClock([ticks[p] if p in chunk else 0 for p in range(N_PROCS)])
        nop = self.nc.sync.nop()
        wait_clock.add_sem_waits(
            nop.ins,
            ScopedClock({None: part}),
            ScopedClock({None: VectorClock(list(observed))}),
        )
        for p in chunk:
            observed[p] = ticks[p]
    drain_inst = self.nc.sync.drain()
    wait_clock.add_sem_waits(
        drain_inst.ins,
        ScopedClock({None: g}),
        ScopedClock({None: VectorClock(list(observed))}),
    )
    self.nc.all_engine_barrier()
    assert self.sems is not None
    popped = self.nc._tile_sem_poison_stack.pop()
    assert popped is self._sem_poison
    self.nc.clear_and_free_semaphores(list(self.sems.allocated().values()))
    self.nc.all_engine_barrier()


tile.TileContext._drain_and_barrier = _patched_drain_and_barrier


def _split_attached_waits(nc):
    n = 0
    for f in nc.m.functions:
        for bb in f.blocks:
            new = []
            for inst in bb.instructions:
                si = getattr(inst, "sync_info", None)
                waits = list(si.on_wait) if (si and si.on_wait) else []
                if waits and not isinstance(inst, mybir.InstEventSemaphore):
                    for k, w in enumerate(waits):
                        n += 1
                        new.append(
                            mybir.InstEventSemaphore(
                                name=f"{inst.name}-w{k}",
                                engine=inst.engine,
                                ins=[],
                                outs=[],
                                sync_info=mybir.SyncInfo(on_wait=[w], on_update=[]),
                            )
                        )
                    si.on_wait = []
                new.append(inst)
            bb.instructions[:] = new
    return n


# const-pack column offsets (cst param, [P, 1184] f32)
CO_ID, CO_U, CO_J, CO_EC = 0, 128, 256, 384
CO_TOK, CO_VS = 512, 528  # [P,16] each
CO_J6 = 544  # [P, 640] iota row 0..639
CST_W = 1184


def build_nc():
    nc = bass.Bass(num_devices=NCORE)
    xts_d = nc.declare_dram_parameter("xts", [P, 2 * NH * T], BF16, isOutput=False)
    xbf_d = nc.declare_dram_parameter("xbf", [T, H], BF16, isOutput=False)
    rw_d = nc.declare_dram_parameter("rws", [P, 2 * NH * E], BF16, isOutput=False)
    w1_d = nc.declare_dram_parameter("w1r", [P, NH * F], BF16, isOutput=False)
    w2_d = nc.declare_dram_parameter("w2r", [P, NF * H], BF16, isOutput=False)
    cst_d = nc.declare_dram_parameter("cst", [P, CST_W], F32, isOutput=False)
    pc_d = nc.declare_dram_parameter("pc", [P, 160], F32, isOutput=False)
    out_d = nc.declare_dram_parameter("out", [SLAB, H], BF16, isOutput=True)

    send_dram = nc.dram_tensor("send_buf", [SEND_FULL, H], BF16)
    recv_dram = nc.dram_tensor("recv_buf", [SEND_ROWS, H], BF16)

    dbg = {}
    if DEBUG:
        for nm, w, dt in [
            ("dbg_lg", NT * E, F32),
            ("dbg_pf", NT * E, F32),
            ("dbg_slot", NT, F32),
            ("dbg_lacc", NC * 3, F32),
            ("dbg_offi", 4, I32),
            ("dbg_xs", NC * H, BF16),
            ("dbg_ysb", NC * H, BF16),
        ]:
            dbg[nm] = nc.declare_dram_parameter(nm, [P, w], dt, isOutput=True)
        dbg["dbg_recv"] = nc.declare_dram_parameter(
            "dbg_recv", [SEND_ROWS, H], BF16, isOutput=True
        )

    raw_ctx = nc.sbuf_tensor("r_offi", [P, 4], I32)
    offi4 = raw_ctx.__enter__()

    tc = tile.TileContext(nc)
    with tc:
        with (
            tc.tile_pool(name="consts", bufs=1) as cb,
            tc.tile_pool(name="weights", bufs=1) as wp,
            tc.tile_pool(name="xq", bufs=2) as xq,
            tc.tile_pool(name="work", bufs=2) as wk,
            tc.tile_pool(name="psum", bufs=2, space="PSUM") as ps,
        ):
            # ---- small const loads first (cheap), then x quarters ----
            cst = cb.tile([P, CST_W], F32, tag="cst")
            nc.sync.dma_start(cst, cst_d[:, :])
            pcs = cb.tile([P, 160], F32, tag="pcs")
            nc.sync.dma_start(pcs, pc_d[:, :])
            rw_t = cb.tile([P, 2, NH, E], BF16, tag="rw")
            nc.sync.dma_start(
                rw_t, rw_d[:, :].rearrange("p (s h e) -> p s h e", h=NH, e=E)
            )

            ident = cst[:, CO_ID : CO_ID + P]
            U = cst[:, CO_U : CO_U + P]
            J = cst[:, CO_J : CO_J + P]
            J640 = cst[:, CO_J6 : CO_J6 + C]
            ecolA = cst[:, CO_EC : CO_EC + P].rearrange("p (i e) -> p i e", e=E)
            tokfA = cst[:, CO_TOK : CO_TOK + NT]
            vslabA = cst[:, CO_VS : CO_VS + NT]
            onehotA = pcs[:, 0:128].rearrange("p (i e) -> p i e", e=E)
            sel0 = pcs[:, 128:144]
            sel1 = pcs[:, 144:160]

            ident_bf = cb.tile([P, P], BF16, tag="ident_bf")
            nc.vector.tensor_copy(ident_bf, ident)
            ones_row = cb.tile([1, P], F32, tag="ones_row")
            nc.vector.memset(ones_row, 1.0)
            ones_col = cb.tile([P, 1], F32, tag="ones_col")
            nc.vector.memset(ones_col, 1.0)
            base_sb = cb.tile([1, 8 * (NT + 1)], F32, tag="base")
            nc.vector.memset(base_sb[:, 0:8], 0.0)

            # psum helper: 5-deep rotation across the two tags
            kk = [0]

            def pst(shape, dtype=F32):
                k = kk[0]
                kk[0] += 1
                if k % 5 < 2:
                    return ps.tile(shape, dtype, tag="sps", bufs=2, space="PSUM",
                                   name=f"sps{k}")
                return ps.tile(shape, dtype, tag="yps", bufs=3, space="PSUM",
                               name=f"yps{k}")

            # PSUM->SBUF copies alternate DVE / Activation (gpsimd can't
            # read PSUM); SBUF-only elementwise work alternates DVE / gpsimd
            def pcopy(k, out, in_):
                if k % 2 == 0:
                    nc.vector.tensor_copy(out, in_)
                else:
                    nc.scalar.activation(out=out, in_=in_, func=AF.Copy)

            cpeng = [nc.vector, nc.gpsimd]

            # ---- x arrives pre-transposed from the host as a bf16 hi/lo
            # pair: xts[p, s, h, t] = split_s(x)[t, 128h + p]. Logits are
            # computed as lgT = rw_T @ xT with three bf16 cross products
            # (hi*HI + hi*LO + lo*HI) accumulated in fp32 PSUM: ~3e-6 abs
            # error, far below the 1.28e-5 top2/top3 tie gap. ----
            xts = []
            for sp in range(2):
                t_ = xq.tile([P, NH, T], BF16, tag=f"xts{sp}", bufs=1,
                             name=f"xts{sp}")
                nc.sync.dma_start(
                    t_,
                    xts_d[:, NH * T * sp : NH * T * (sp + 1)].rearrange(
                        "p (h t) -> p h t", t=T
                    ),
                )
                xts.append(t_)

            # ---- weights resident (issued after x quarters so x wins
            # the DMA bandwidth race; transfers overlap routing compute) ----
            w1sb = wp.tile([P, NH, F], BF16, tag="w1sb", name="w1sb")
            nc.sync.dma_start(w1sb, w1_d[:, :].rearrange("p (c f) -> p c f", f=F))
            w2sb = wp.tile([P, NF, H], BF16, tag="w2sb", name="w2sb")
            nc.sync.dma_start(w2sb, w2_d[:, :].rearrange("p (c h) -> p c h", h=H))

            # ---- logits: lgT[e, t] accumulated in 4 psum quarters of 512
            # tokens; product-outer order so hi*HI starts as soon as the hi
            # half of x lands ----
            lgA = cb.tile([P, NT, E], F32, tag="lgA")
            valsA = cb.tile([P, NT, 8], F32, tag="valsA")
            idxA = cb.tile([P, NT, 8], U32, tag="idxA")
            lgT = [
                ps.tile([E, 1024], F32, tag="yps", bufs=3, space="PSUM",
                        name=f"lgT{q}")
                for q in range(2)
            ]
            prods = [(0, 0), (1, 0), (0, 1)]  # (rw split, x split)
            for pi, (sa, sb) in enumerate(prods):
                for q4 in range(4):
                    for h in range(NH):
                        nc.tensor.matmul(
                            lgT[q4 // 2][:, 512 * (q4 % 2) : 512 * (q4 % 2 + 1)],
                            lhsT=rw_t[:, sa, h, :],
                            rhs=xts[sb][:, h, 512 * q4 : 512 * (q4 + 1)],
                            start=(pi == 0 and h == 0),
                            stop=(pi == len(prods) - 1 and h == NH - 1),
                        )
            # reuses the x_lo slot (dead after the logits products)
            lgS = xq.tile([E, T], F32, tag="xts1", bufs=1, name="lgS")
            nc.vector.tensor_copy(lgS[:, 0:1024], lgT[0])
            nc.vector.tensor_copy(lgS[:, 1024:2048], lgT[1])
            for i in range(NT):
                tp = pst([P, E])
                nc.tensor.transpose(
                    tp, lgS[:, P * i : P * (i + 1)], ident[0:E, 0:E]
                )
                nc.vector.tensor_copy(lgA[:, i, :], tp)
                nc.vector.max(out=valsA[:, i, :], in_=lgA[:, i, :])
                nc.vector.max_index(
                    out=idxA[:, i, :], in_max=valsA[:, i, :], in_values=lgA[:, i, :]
                )

            # ---- batched top-2 weights + masks ----
            idxfA = cb.tile([P, NT, 8], F32, tag="idxfA")
            nc.vector.tensor_copy(idxfA, idxA)
            eq1A = cb.tile([P, NT, E], F32, tag="eq1A")
            eq2A = cb.tile([P, NT, E], F32, tag="eq2A")
            M_A = cb.tile([P, NT, E], F32, tag="M_A")
            dA = wk.tile([P, NT], F32, tag="dA")
            nc.vector.tensor_tensor(
                out=dA, in0=valsA[:, :, 1], in1=valsA[:, :, 0], op=OP.subtract
            )
            eA = wk.tile([P, NT], F32, tag="eA")
            nc.scalar.activation(out=eA, in_=dA, func=AF.Exp)
            smA = wk.tile([P, NT], F32, tag="smA")
            nc.vector.tensor_scalar_add(smA, eA, 1.0)
            w1nA = wk.tile([P, NT], F32, tag="w1nA")
            nc.vector.reciprocal(w1nA, smA)
            w2nA = wk.tile([P, NT], F32, tag="w2nA")
            nc.vector.tensor_tensor(out=w2nA, in0=eA, in1=w1nA, op=OP.mult)
            nc.vector.tensor_tensor(
                out=eq1A,
                in0=ecolA,
                in1=idxfA[:, :, 0:1].to_broadcast([P, NT, E]),
                op=OP.is_equal,
            )
            nc.vector.tensor_tensor(
                out=eq2A,
                in0=ecolA,
                in1=idxfA[:, :, 1:2].to_broadcast([P, NT, E]),
                op=OP.is_equal,
            )
            nc.vector.tensor_tensor(out=M_A, in0=eq1A, in1=eq2A, op=OP.add)
            M_flat = M_A.rearrange("p i e -> p (i e)")

            # ---- counts + base prefix chain ----
            cnt_ps = pst([1, NT * E])
            nc.tensor.matmul(cnt_ps, lhsT=ones_col, rhs=M_flat, start=True, stop=True)
            cntA = cb.tile([1, NT * E], F32, tag="cntA")
            nc.vector.tensor_copy(cntA, cnt_ps)
            for i in range(NT):
                nc.vector.tensor_tensor(
                    out=base_sb[:, 8 * (i + 1) : 8 * (i + 2)],
                    in0=base_sb[:, 8 * i : 8 * (i + 1)],
                    in1=cntA[:, 8 * i : 8 * (i + 1)],
                    op=OP.add,
                )

            # ---- batched prefix matmuls ----
            base_slab = cb.tile([1, NT * E], F32, tag="base_slab")
            for j in range(NT // 2):
                nc.vector.tensor_copy(
                    base_slab[:, 16 * j : 16 * j + 8], base_sb[:, 16 * j : 16 * j + 8]
                )
                nc.vector.tensor_copy(
                    base_slab[:, 16 * j + 8 : 16 * j + 16],
                    base_sb[:, 16 * j : 16 * j + 8],
                )
            base_dif = cb.tile([1, NT * E], F32, tag="base_dif")
            nc.vector.tensor_tensor(
                out=base_dif, in0=base_sb[:, 0:128], in1=base_slab, op=OP.subtract
            )
            PslabA = cb.tile([P, NT, E], F32, tag="PslabA")
            PfullA = cb.tile([P, NT, E], F32, tag="PfullA")
            psl_ps = pst([P, NT * E])
            nc.tensor.matmul(psl_ps, lhsT=U, rhs=M_flat, start=True, stop=False)
            nc.tensor.matmul(
                psl_ps, lhsT=ones_row, rhs=base_dif, start=False, stop=True
            )
            nc.vector.tensor_copy(PslabA.rearrange("p i e -> p (i e)"), psl_ps)
            pfu_ps = pst([P, NT * E])
            nc.tensor.matmul(pfu_ps, lhsT=U, rhs=M_flat, start=True, stop=False)
            nc.tensor.matmul(
                pfu_ps, lhsT=ones_row, rhs=base_sb[:, 0:128], start=False, stop=True
            )
            nc.vector.tensor_copy(PfullA.rearrange("p i e -> p (i e)"), pfu_ps)

            # ---- slot/weight/offset algebra (batched) ----
            GA = wk.tile([P, NT, E], F32, tag="GA")  # CAP*e + pos_slab
            gec = wk.tile([P, NT, E], F32, tag="gec")
            nc.vector.tensor_scalar(gec, ecolA, float(CAP), None, op0=OP.mult)
            nc.vector.tensor_tensor(out=GA, in0=PslabA, in1=gec, op=OP.add)
            t1 = wk.tile([P, NT, E], F32, tag="t1")
            nc.vector.tensor_tensor(out=t1, in0=GA, in1=eq1A, op=OP.mult)
            off1A = wk.tile([P, NT], F32, tag="off1A")
            nc.vector.reduce_sum(off1A, t1, axis=AX.X)
            t2 = wk.tile([P, NT, E], F32, tag="t2")
            nc.vector.tensor_tensor(out=t2, in0=GA, in1=eq2A, op=OP.mult)
            off2A = wk.tile([P, NT], F32, tag="off2A")
            nc.vector.reduce_sum(off2A, t2, axis=AX.X)

            # combine-side offsets for this core's slab (columns: 2a+k),
            # written to a raw (non-pool) SBUF tensor so the raw tail can
            # reference it without symbolic-AP trouble
            for a, sel in ((0, sel0), (1, sel1)):
                for k, offk in ((0, off1A), (1, off2A)):
                    tsel = wk.tile([P, NT], F32, tag="tsel", bufs=4, name=f"ts{a}{k}")
                    nc.vector.tensor_tensor(out=tsel, in0=offk, in1=sel, op=OP.mult)
                    osel = wk.tile([P, 1], F32, tag="osel", bufs=4, name=f"os{a}{k}")
                    nc.vector.reduce_sum(osel, tsel, axis=AX.X)
                    nc.vector.tensor_copy(offi4[:, 2 * a + k : 2 * a + k + 1], osel)

            selM = wk.tile([P, NT, E], F32, tag="selM")
            nc.vector.tensor_tensor(out=selM, in0=M_A, in1=onehotA, op=OP.mult)
            m_cA = wk.tile([P, NT], F32, tag="m_cA")
            nc.vector.reduce_sum(m_cA, selM, axis=AX.X)
            selP = wk.tile([P, NT, E], F32, tag="selP")
            nc.vector.tensor_tensor(out=selP, in0=PfullA, in1=onehotA, op=OP.mult)
            slot_cA = wk.tile([P, NT], F32, tag="slot_cA")
            nc.vector.reduce_sum(slot_cA, selP, axis=AX.X)
            selS = wk.tile([P, NT, E], F32, tag="selS")
            nc.vector.tensor_tensor(out=selS, in0=PslabA, in1=onehotA, op=OP.mult)
            pos_cA = wk.tile([P, NT], F32, tag="pos_cA")
            nc.vector.reduce_sum(pos_cA, selS, axis=AX.X)
            v_cA = wk.tile([P, NT], F32, tag="v_cA")
            nc.vector.tensor_tensor(out=v_cA, in0=vslabA, in1=pos_cA, op=OP.subtract)
            Wa = wk.tile([P, NT, E], F32, tag="Wa")
            nc.vector.tensor_tensor(
                out=Wa,
                in0=eq1A,
                in1=w1nA.unsqueeze(2).to_broadcast([P, NT, E]),
                op=OP.mult,
            )
            Wb = wk.tile([P, NT, E], F32, tag="Wb")
            nc.vector.tensor_tensor(
                out=Wb,
                in0=eq2A,
                in1=w2nA.unsqueeze(2).to_broadcast([P, NT, E]),
                op=OP.mult,
            )
            Ws = wk.tile([P, NT, E], F32, tag="Ws")
            nc.vector.tensor_tensor(out=Ws, in0=Wa, in1=Wb, op=OP.add)
            selW = wk.tile([P, NT, E], F32, tag="selW")
            nc.vector.tensor_tensor(out=selW, in0=Ws, in1=onehotA, op=OP.mult)
            w_cA = wk.tile([P, NT], F32, tag="w_cA")
            nc.vector.reduce_sum(w_cA, selW, axis=AX.X)
            nmA = wk.tile([P, NT], F32, tag="nmA")
            nc.vector.tensor_scalar(nmA, m_cA, -BIG, BIG, op0=OP.mult, op1=OP.add)
            slot_mA = wk.tile([P, NT], F32, tag="slot_mA")
            nc.vector.tensor_tensor(out=slot_mA, in0=slot_cA, in1=nmA, op=OP.add)
            payloadA = cb.tile([P, NT, 3], F32, tag="payloadA")
            nc.vector.tensor_copy(payloadA[:, :, 0], tokfA)
            nc.vector.tensor_copy(payloadA[:, :, 1], w_cA)
            nc.vector.tensor_copy(payloadA[:, :, 2], v_cA)

            # ---- on-chip compaction: lacc[p, jt] = payload of slot 128*jt+p.
            # One wide fp16 one-hot row per token tile (all values involved
            # are exactly representable in fp16), then 16-step accumulating
            # selection matmuls per compact tile. ----
            F16 = mybir.dt.float16
            payload_h = cb.tile([P, NT, 3], F16, tag="payload_h")
            nc.vector.tensor_copy(payload_h, payloadA)
            lacc = cb.tile([P, NC, 3], F32, tag="lacc")
            cp_ps = [pst([P, 3]) for _ in range(NC)]
            for i in range(NT):
                eq = wk.tile([P, C], F16, tag="eqt", bufs=3, name=f"eq{i}")
                nc.vector.tensor_tensor(
                    out=eq,
                    in0=J640,
                    in1=slot_mA[:, i : i + 1].to_broadcast([P, C]),
                    op=OP.is_equal,
                )
                for jt in range(NC):
                    nc.tensor.matmul(
                        cp_ps[jt],
                        lhsT=eq[:, P * jt : P * (jt + 1)],
                        rhs=payload_h[:, i, :],
                        start=(i == 0),
                        stop=(i == NT - 1),
                    )
            for jt in range(NC):
                nc.vector.tensor_copy(lacc[:, jt, :], cp_ps[jt])

            idx_all = cb.tile([P, NC], I32, tag="idx_all")
            nc.vector.tensor_copy(idx_all, lacc[:, :, 0])
            scat_f = wk.tile([P, NC], F32, tag="scat_f")
            nc.vector.tensor_scalar(
                scat_f, lacc[:, :, 2], -1.0, float(SEND_FULL - 1),
                op0=OP.mult, op1=OP.add,
            )
            scat_all = cb.tile([P, NC], I32, tag="scat_all")
            nc.vector.tensor_copy(scat_all, scat_f)

            # ---- gather compact tokens (bf16) + transpose ----
            xs_all = cb.tile([P, NC, H], BF16, tag="xs_all")
            xsT = cb.tile([P, NH, C], BF16, tag="xsT")
            for (j0, j1) in WAVES:
                for j in range(j0, j1):
                    nc.gpsimd.indirect_dma_start(
                        out=xs_all[:, j, :],
                        out_offset=None,
                        in_=xbf_d[:, :],
                        in_offset=IndirectOffsetOnAxis(ap=idx_all[:, j : j + 1], axis=0),
                        bounds_check=T - 1,
                        oob_is_err=False,
                    )
                for j in range(j0, j1):
                    for h in range(NH):
                        tpb = pst([P, P], BF16)
                        nc.tensor.transpose(
                            tpb, xs_all[:, j, P * h : P * (h + 1)], ident_bf
                        )
                        pcopy(j * NH + h, xsT[:, h, P * j : P * (j + 1)], tpb)

            # ---- compact MLP in waves ----
            hT_all = cb.tile([P, NF, C], BF16, tag="bigslot", name="hT_all")
            ysb_all = cb.tile([P, NC, H], BF16, tag="ysb_all")
            for (j0, j1) in WAVES:
                W = (j1 - j0) * P
                y_ps = [
                    ps.tile([P, H], F32, tag="yps", bufs=3, space="PSUM",
                            name=f"y{tt}")
                    for tt in range(j0, j1)
                ]
                # software pipeline: w1(f+1) issues before w2(f) so the
                # silu(f) -> w2(f) latency hides behind w1(f+1) compute
                def w1_stage(f):
                    hps = ps.tile([P, W], F32, tag="sps", bufs=2, space="PSUM",
                                  name=f"h{j0}_{f}")
                    for h in range(NH):
                        nc.tensor.matmul(
                            hps,
                            lhsT=w1sb[:, h, P * f : P * (f + 1)],
                            rhs=xsT[:, h, P * j0 : P * j1],
                            start=(h == 0),
                            stop=(h == NH - 1),
                        )
                    nc.scalar.activation(
                        out=hT_all[:, f, P * j0 : P * j1], in_=hps, func=AF.Silu
                    )

                def w2_stage(f):
                    for ti, tt in enumerate(range(j0, j1)):
                        nc.tensor.matmul(
                            y_ps[ti][:, 0:512],
                            lhsT=hT_all[:, f, P * tt : P * (tt + 1)],
                            rhs=w2sb[:, f, 0:512],
                            start=(f == 0),
                            stop=(f == NF - 1),
                        )
                        nc.tensor.matmul(
                            y_ps[ti][:, 512:768],
                            lhsT=hT_all[:, f, P * tt : P * (tt + 1)],
                            rhs=w2sb[:, f, 512:768],
                            start=(f == 0),
                            stop=(f == NF - 1),
                        )

                w1_stage(0)
                for f in range(NF):
                    if f + 1 < NF:
                        w1_stage(f + 1)
                    w2_stage(f)
                for ti, tt in enumerate(range(j0, j1)):
                    nc.vector.tensor_scalar(
                        ysb_all[:, tt, :], y_ps[ti][:, 0:H], lacc[:, tt, 1:2],
                        None, op0=OP.mult,
                    )
                for tt in range(j0, j1):
                    nc.gpsimd.indirect_dma_start(
                        out=send_dram[:, :],
                        out_offset=IndirectOffsetOnAxis(
                            ap=scat_all[:, tt : tt + 1], axis=0
                        ),
                        in_=ysb_all[:, tt, :],
                        in_offset=None,
                        bounds_check=SEND_FULL - 1,
                        oob_is_err=False,
                    )

            if DEBUG:
                nc.sync.dma_start(dbg["dbg_lg"][:, :], lgA.rearrange("p i e -> p (i e)"))
                nc.sync.dma_start(
                    dbg["dbg_pf"][:, :], PfullA.rearrange("p i e -> p (i e)")
                )
                nc.sync.dma_start(dbg["dbg_slot"][:, :], slot_mA)
                nc.sync.dma_start(
                    dbg["dbg_lacc"][:, :], lacc.rearrange("p a c -> p (a c)")
                )
                nc.sync.dma_start(dbg["dbg_offi"][:, :], offi4[:, :])
                nc.sync.dma_start(
                    dbg["dbg_xs"][:, :], xs_all.rearrange("p a c -> p (a c)")
                )
                nc.sync.dma_start(
                    dbg["dbg_ysb"][:, :], ysb_all.rearrange("p a c -> p (a c)")
                )

    # ---- raw tail: AllToAll + owner-side combine (pair-sum via CCE add
    # on the second gather; bf16 out, host converts to f32) ----
    with (
        nc.semaphore("fin_sem") as fsem,
        nc.sbuf_tensor("r_gall", [P, 2, H], BF16) as gall,
        nc.Block() as blk,
    ):

        @blk.gpsimd
        def _(g: bass.BassEngine):
            g.collective_compute(
                "AllToAll",
                OP.bypass,
                replica_groups=[list(range(NCORE))],
                ins=[send_dram[0:SEND_ROWS, :].opt()],
                outs=[recv_dram[:, :].opt()],
            ).then_inc(fsem, 1)
            g.wait_ge(fsem, 1)
            for a in range(2):
                g.indirect_dma_start(
                    out=gall[:, a, :],
                    out_offset=None,
                    in_=recv_dram[:, :],
                    in_offset=IndirectOffsetOnAxis(
                        ap=offi4[:, 2 * a : 2 * a + 1], axis=0
                    ),
                    bounds_check=SEND_ROWS - 1,
                    oob_is_err=False,
                ).then_inc(fsem, 16)
            g.wait_ge(fsem, 33)
            for a in range(2):
                g.indirect_dma_start(
                    out=gall[:, a, :],
                    out_offset=None,
                    in_=recv_dram[:, :],
                    in_offset=IndirectOffsetOnAxis(
                        ap=offi4[:, 2 * a + 1 : 2 * a + 2], axis=0
                    ),
                    bounds_check=SEND_ROWS - 1,
                    oob_is_err=False,
                    compute_op=OP.add,
                ).then_inc(fsem, 16)

        @blk.sync
        def _(s: bass.BassEngine):
            s.wait_ge(fsem, 65)
            s.dma_start(
                out_d[:, :].rearrange("(a p) h -> p a h", p=P), gall[:, :, :]
            ).then_inc(fsem, 16)
            if DEBUG:
                s.dma_start(dbg["dbg_recv"][:, :], recv_dram[:, :]).then_inc(fsem, 16)
                s.wait_ge(fsem, 97)
            else:
                s.wait_ge(fsem, 81)

    _split_attached_waits(nc)
    return nc


def make_in_maps(x, router_w, w1, w2):
    import ml_dtypes

    bf16 = ml_dtypes.bfloat16
    x = np.ascontiguousarray(np.asarray(x, np.float32))
    rw = np.ascontiguousarray(np.asarray(router_w, np.float32))
    w1 = np.asarray(w1, np.float32)
    w2 = np.asarray(w2, np.float32)

    # pre-transposed split-bf16 routing x: xts[p, (s, h, t)] = s(x)[t, 128h+p]
    x_hi = x.astype(bf16).astype(np.float32)
    x_lo = (x - x_hi).astype(bf16)
    def _xt(v):
        return v.astype(bf16).reshape(T, NH, P).transpose(2, 1, 0)
    xts = np.ascontiguousarray(
        np.stack([_xt(x_hi), _xt(x_lo)], axis=1).reshape(P, 2 * NH * T)
    )
    xbf = np.ascontiguousarray(x.astype(bf16))
    rw_hi = rw.astype(bf16).astype(np.float32)
    rw_lo = (rw - rw_hi).astype(bf16)
    def _rwt(v):
        return v.astype(bf16).reshape(NH, P, E).transpose(1, 0, 2)
    rws = np.ascontiguousarray(
        np.stack([_rwt(rw_hi), _rwt(rw_lo)], axis=1).reshape(P, 2 * NH * E)
    )

    cst = np.zeros((P, CST_W), np.float32)
    cst[:, CO_J6 : CO_J6 + C] = np.tile(np.arange(C, dtype=np.float32), (P, 1))
    cst[:, CO_ID : CO_ID + P] = np.eye(P, dtype=np.float32)
    cst[:, CO_U : CO_U + P] = np.triu(np.ones((P, P), np.float32), 1)
    cst[:, CO_J : CO_J + P] = np.tile(np.arange(P, dtype=np.float32), (P, 1))
    cst[:, CO_EC : CO_EC + P] = np.tile(
        np.arange(E, dtype=np.float32), (P, NT)
    )
    cst[:, CO_TOK : CO_TOK + NT] = (
        np.arange(P)[:, None] + P * np.arange(NT)[None, :]
    ).astype(np.float32)
    cst[:, CO_VS : CO_VS + NT] = np.tile(
        (float(SEND_FULL - 1) - CAP * (np.arange(NT) >> 1))[None, :].astype(
            np.float32
        ),
        (P, 1),
    )

    in_maps = []
    for c in range(NCORE):
        oh = np.zeros((P, NT, E), np.float32)
        oh[:, :, c] = 1.0
        pc = np.zeros((P, 160), np.float32)
        pc[:, 0:128] = oh.reshape(P, NT * E)
        pc[:, 128 + 2 * c] = 1.0        # sel0: tile 2c
        pc[:, 144 + 2 * c + 1] = 1.0    # sel1: tile 2c+1
        w1r = np.ascontiguousarray(
            w1[c].astype(bf16).reshape(NH, P, F).transpose(1, 0, 2).reshape(P, NH * F)
        )
        w2r = np.ascontiguousarray(
            w2[c].astype(bf16).reshape(NF, P, H).transpose(1, 0, 2).reshape(P, NF * H)
        )
        in_maps.append(
            {
                "xts": xts,
                "xbf": xbf,
                "rws": rws,
                "w1r": w1r,
                "w2r": w2r,
                "cst": cst,
                "pc": pc,
            }
        )
    return in_maps


def gather_output(results):
    return np.concatenate(
        [results[c]["out"].astype(np.float32) for c in range(NCORE)], axis=0
    )


def kernel(x, router_w, w1, w2):
    from concourse.bass_utils import run_bass_kernel_spmd

    nc = build_nc()
    in_maps = make_in_maps(x, router_w, w1, w2)
    res = run_bass_kernel_spmd(nc, in_maps, list(range(NCORE)))
    return gather_output(res.results)
